# revision 6
# baseline (speedup 1.0000x reference)
"""GQA attention (llama3-style RoPE, causal) on 8 trn2 NeuronCores.

Sharding: tensor-parallel over KV-head groups. Core i owns kv-head i and its
4 query heads: wq[:, i*512:(i+1)*512], wk/wv[:, i*128:(i+1)*128], and the
matching row-slice wo[i*512:(i+1)*512, :]. After the partial o_proj, a
ReduceScatter(add) over the sequence dim leaves core i with output rows
[i*256, (i+1)*256); the host concatenates the shards.

On-device layout (per core): everything is computed transposed-by-design so
no PE transposes are needed in the hot path:
  qT/kT  [d=128, T]   = wq/wk-tile.T @ xT          (lhsT=weight, rhs=xT)
  vT     [d, T]       -> PE-transposed to v [T, d] (16 small transposes)
  sT     [tk, tq]     = k @ qT                     (lhsT=kT-tile, rhs=qT)
  pT     = exp(sT/sqrt(d)) * causal-mask
  l(row) [1, tq]      = ones.T @ pT                (M=1 matmul, PSUM-accum)
  oT     [d, tq]      = v.T @ pT                   (PSUM-accum over tk)
  o_norm = oT * (1/l) partition-broadcast
  partial[t, :]       = out_heads @ wo_i           (lhsT=oT-tile, rhs=wo)
All matmuls run as float32r (1 cycle/row vs 4 for fp32; ~1e-4 rel err).
"""

import numpy as np

H, KV, HD, HID = 32, 8, 128, 4096
T = 2048
N_CORES = 8
QH = H // KV            # 4 query heads per core
DQ = QH * HD            # 512
KT = HID // 128         # 32 contraction tiles for projections
TN = T // 128           # 16 sequence tiles
G = 4                   # tq groups of 512
GW = T // G             # 512
TS = T // N_CORES       # 256 output rows per core after ReduceScatter

THETA, FACTOR, HI_FF, LO_FF, ORIG_MAX = 500000.0, 8.0, 4.0, 1.0, 8192

_CACHE = {}


def _rope_tables():
    inv = 1.0 / (THETA ** (np.arange(0, HD, 2, dtype=np.float64) / HD))
    wavelen = 2.0 * np.pi / inv
    low_wl = ORIG_MAX / LO_FF
    high_wl = ORIG_MAX / HI_FF
    smooth = (ORIG_MAX / wavelen - LO_FF) / (HI_FF - LO_FF)
    scaled = np.where(wavelen > low_wl, inv / FACTOR, inv)
    mid = (wavelen <= low_wl) & (wavelen >= high_wl)
    scaled = np.where(mid, (1 - smooth) * inv / FACTOR + smooth * inv, scaled)
    inv32 = scaled.astype(np.float32)
    pos = np.arange(T, dtype=np.float32)
    freqs = pos[:, None] * inv32[None, :]          # [T, 64]
    emb = np.concatenate([freqs, freqs], axis=-1)  # [T, 128]
    cosT = np.ascontiguousarray(np.cos(emb).T)     # [128, T]
    sinT = np.ascontiguousarray(np.sin(emb).T)
    return cosT, sinT


def _causal_masks():
    # pT tile is [tk(part) 128, tq(free) 512]; within a tq-group the diagonal
    # tile sits at block v (=tk_tile - 4*g). keep where tq >= tk.
    tri = np.triu(np.ones((128, 128), dtype=np.float32))
    masks = np.zeros((4, 128, 512), dtype=np.float32)
    for v in range(4):
        for c in range(4):
            if c > v:
                masks[v, :, c * 128:(c + 1) * 128] = 1.0
            elif c == v:
                masks[v, :, c * 128:(c + 1) * 128] = tri
    return masks


def _build_program():
    import concourse.bacc as bacc
    import concourse.mybir as mybir
    from concourse.tile import TileContext

    f32 = mybir.dt.float32
    f32r = mybir.dt.float32r
    EXPF = mybir.ActivationFunctionType.Exp

    nc = bacc.Bacc("TRN2", target_bir_lowering=False, debug=False,
                   num_devices=N_CORES)

    xT = nc.dram_tensor("xT", [HID, T], f32, kind="ExternalInput")
    wqd = nc.dram_tensor("wq", [HID, DQ], f32, kind="ExternalInput")
    wkd = nc.dram_tensor("wk", [HID, HD], f32, kind="ExternalInput")
    wvd = nc.dram_tensor("wv", [HID, HD], f32, kind="ExternalInput")
    wod = nc.dram_tensor("wo", [DQ, HID], f32, kind="ExternalInput")
    cosd = nc.dram_tensor("cosT", [HD, T], f32, kind="ExternalInput")
    sind = nc.dram_tensor("sinT", [HD, T], f32, kind="ExternalInput")
    maskd = nc.dram_tensor("masks", [4, HD, GW], f32, kind="ExternalInput")
    identd = nc.dram_tensor("ident", [128, 128], f32, kind="ExternalInput")
    onesd = nc.dram_tensor("ones", [128, 1], f32, kind="ExternalInput")
    outd = nc.dram_tensor("out", [TS, HID], f32, kind="ExternalOutput")

    partial = nc.dram_tensor("partial", [T, HID], f32)
    rs_out = nc.dram_tensor("rs_out", [TS, HID], f32)

    def r(ap):
        return ap.bitcast(f32r)

    with TileContext(nc) as tc:
        with (
            tc.tile_pool(name="const", bufs=1) as cpool,
            tc.tile_pool(name="qkv", bufs=1) as qkv,
            tc.tile_pool(name="obuf", bufs=4) as obp,
        ):
            # ---- constants ----
            cos = cpool.tile([HD, T], f32, tag="cos")
            sin = cpool.tile([HD, T], f32, tag="sin")
            nc.sync.dma_start(cos[:], cosd[:])
            nc.sync.dma_start(sin[:], sind[:])
            ident = cpool.tile([128, 128], f32r, tag="ident")
            nc.sync.dma_start(ident[:], r(identd[:]))
            ones = cpool.tile([128, 1], f32r, tag="ones")
            nc.sync.dma_start(ones[:], r(onesd[:]))
            # persistent activations
            qT = [qkv.tile([128, T], f32r, tag=f"qT{h}", name=f"qT{h}") for h in range(QH)]
            kTt = qkv.tile([128, T], f32r, tag="kT")
            vsb = qkv.tile([128, T], f32r, tag="vsb")  # v tiles side by side

            # ---- phase B: projections + RoPE (+ v transpose) ----
            with (
                tc.tile_pool(name="wq", bufs=1) as wqp,
                tc.tile_pool(name="stream", bufs=3) as stp,
                tc.tile_pool(name="tmp", bufs=2) as tmp,
                tc.tile_pool(name="ppsum", bufs=1, space="PSUM") as pps,
                tc.tile_pool(name="trpsum", bufs=2, space="PSUM") as trp,
            ):
                wq_t = []
                for k in range(KT):
                    wt = wqp.tile([128, DQ], f32r, tag=f"wq{k}", name=f"wq{k}")
                    nc.sync.dma_start(wt[:], r(wqd[k * 128:(k + 1) * 128, :]))
                    wq_t.append(wt)

                for g in range(G):
                    gs = slice(g * GW, (g + 1) * GW)
                    qps = [pps.tile([128, GW], f32, tag=f"qp{h}", name=f"qp{h}_{g}") for h in range(QH)]
                    kps = pps.tile([128, GW], f32, tag="kp")
                    vps = pps.tile([128, GW], f32, tag="vp")
                    for k in range(KT):
                        xt = stp.tile([128, GW], f32r, tag="xt")
                        nc.sync.dma_start(xt[:], r(xT[k * 128:(k + 1) * 128, gs]))
                        wkt = stp.tile([128, HD], f32r, tag="wkt")
                        nc.sync.dma_start(wkt[:], r(wkd[k * 128:(k + 1) * 128, :]))
                        wvt = stp.tile([128, HD], f32r, tag="wvt")
                        nc.sync.dma_start(wvt[:], r(wvd[k * 128:(k + 1) * 128, :]))
                        st = (k == 0)
                        sp = (k == KT - 1)
                        for h in range(QH):
                            nc.tensor.matmul(qps[h][:], wq_t[k][:, h * 128:(h + 1) * 128],
                                             xt[:], start=st, stop=sp)
                        nc.tensor.matmul(kps[:], wkt[:], xt[:], start=st, stop=sp)
                        nc.tensor.matmul(vps[:], wvt[:], xt[:], start=st, stop=sp)

                    # RoPE drain for q heads and k; v transpose
                    for h in range(QH + 1):
                        src = qps[h] if h < QH else kps
                        dst = qT[h] if h < QH else kTt
                        t1 = tmp.tile([128, GW], f32, tag="t1")
                        nc.vector.tensor_mul(t1[:], src[:], cos[:, gs])
                        rot = tmp.tile([128, GW], f32, tag="rot")
                        nc.scalar.mul(rot[0:64, :], src[64:128, :], -1.0)
                        nc.scalar.copy(rot[64:128, :], src[0:64, :])
                        rot2 = tmp.tile([128, GW], f32, tag="rot2")
                        nc.vector.tensor_mul(rot2[:], rot[:], sin[:, gs])
                        nc.vector.tensor_add(dst[:, gs].bitcast(f32r), t1[:], rot2[:])

                    vTt = tmp.tile([128, GW], f32r, tag="vT")
                    nc.vector.tensor_copy(vTt[:], vps[:])
                    for ts in range(4):
                        tp = trp.tile([128, 128], f32r, tag="trp")
                        nc.tensor.transpose(tp[:], vTt[:, ts * 128:(ts + 1) * 128], ident[:])
                        nc.vector.tensor_copy(
                            vsb[:, (4 * g + ts) * 128:(4 * g + ts + 1) * 128].bitcast(f32r),
                            tp[:])

            # ---- phase C: attention ----
            scale = float(1.0 / np.sqrt(HD))
            with (
                tc.tile_pool(name="mask", bufs=1) as mpool,
                tc.tile_pool(name="oT", bufs=1) as otp,
                tc.tile_pool(name="wos", bufs=2) as wop,
                tc.tile_pool(name="pt", bufs=4) as ptp,
                tc.tile_pool(name="norm", bufs=2) as nrm,
                tc.tile_pool(name="apsum", bufs=2, space="PSUM") as aps,
                tc.tile_pool(name="apsum1", bufs=1, space="PSUM") as aps1,
                tc.tile_pool(name="opsum", bufs=4, space="PSUM") as opsum,
            ):
                mtiles = []
                for v in range(4):
                    mt = mpool.tile([HD, GW], f32r, tag=f"mask{v}", name=f"mask{v}")
                    nc.sync.dma_start(mt[:], r(maskd[v]))
                    mtiles.append(mt)
                oT = [otp.tile([128, T], f32r, tag=f"oT{h}", name=f"oT{h}") for h in range(QH)]
                for h in range(QH):
                    for g in range(G):
                        gs = slice(g * GW, (g + 1) * GW)
                        nj = 4 * g + 4
                        ops_ = aps1.tile([128, GW], f32, tag="op")
                        lps = aps1.tile([1, GW], f32, tag="lp")
                        for j in range(nj):
                            sps = aps.tile([128, GW], f32, tag="sp")
                            nc.tensor.matmul(sps[:], kTt[:, j * 128:(j + 1) * 128],
                                             qT[h][:, gs], start=True, stop=True)
                            pt = ptp.tile([128, GW], f32r, tag="pt")
                            nc.scalar.activation(pt[:], sps[:], EXPF, scale=scale)
                            if j >= 4 * g:
                                nc.vector.tensor_mul(pt[:], pt[:], mtiles[j - 4 * g][:])
                            nc.tensor.matmul(lps[:], ones[:], pt[:],
                                             start=(j == 0), stop=(j == nj - 1))
                            nc.tensor.matmul(ops_[:], vsb[:, j * 128:(j + 1) * 128],
                                             pt[:], start=(j == 0), stop=(j == nj - 1))
                        ls = nrm.tile([1, GW], f32, tag="ls")
                        nc.vector.reciprocal(ls[:], lps[:])
                        lb = nrm.tile([128, GW], f32, tag="lb")
                        nc.gpsimd.partition_broadcast(lb[:], ls[:])
                        nc.vector.tensor_mul(oT[h][:, gs].bitcast(f32r), ops_[:], lb[:])

                # ---- phase D: o_proj (n-outer so each wo chunk loads once) ----
                for n in range(HID // 512):
                    wo_c = []
                    for f in range(QH):
                        wc = wop.tile([128, 512], f32r, tag=f"woc{f}", name=f"woc{f}_{n}")
                        nc.sync.dma_start(
                            wc[:], r(wod[f * 128:(f + 1) * 128, n * 512:(n + 1) * 512]))
                        wo_c.append(wc)
                    for t in range(TN):
                        ops_ = opsum.tile([128, 512], f32, tag="oproj")
                        for f in range(QH):
                            nc.tensor.matmul(ops_[:], oT[f][:, t * 128:(t + 1) * 128],
                                             wo_c[f][:],
                                             start=(f == 0), stop=(f == QH - 1))
                        ob = obp.tile([128, 512], f32, tag="ob")
                        nc.vector.tensor_copy(ob[:], ops_[:])
                        nc.sync.dma_start(
                            partial[t * 128:(t + 1) * 128, n * 512:(n + 1) * 512], ob[:])

            # ---- phase E: ReduceScatter + output ----
            nc.gpsimd.collective_compute(
                "ReduceScatter", mybir.AluOpType.add,
                replica_groups=[list(range(N_CORES))],
                ins=[partial[:]], outs=[rs_out[:]],
            )
            nc.sync.dma_start(outd[:], rs_out[:])

    nc.compile()
    return nc


def _get_program():
    if "nc" not in _CACHE:
        _CACHE["nc"] = _build_program()
    return _CACHE["nc"]


def kernel(x, wq, wk, wv, wo):
    from concourse.bass_utils import run_bass_kernel_spmd

    nc = _get_program()

    x2 = np.asarray(x, dtype=np.float32).reshape(T, HID)
    xT = np.ascontiguousarray(x2.T)
    cosT, sinT = _rope_tables()
    masks = _causal_masks()
    ident = np.eye(128, dtype=np.float32)
    ones = np.ones((128, 1), dtype=np.float32)

    wq = np.asarray(wq, dtype=np.float32)
    wk = np.asarray(wk, dtype=np.float32)
    wv = np.asarray(wv, dtype=np.float32)
    wo = np.asarray(wo, dtype=np.float32)

    in_maps = []
    for i in range(N_CORES):
        in_maps.append({
            "xT": xT,
            "wq": np.ascontiguousarray(wq[:, i * DQ:(i + 1) * DQ]),
            "wk": np.ascontiguousarray(wk[:, i * HD:(i + 1) * HD]),
            "wv": np.ascontiguousarray(wv[:, i * HD:(i + 1) * HD]),
            "wo": np.ascontiguousarray(wo[i * DQ:(i + 1) * DQ, :]),
            "cosT": cosT,
            "sinT": sinT,
            "masks": masks,
            "ident": ident,
            "ones": ones,
        })

    _CACHE["last_in_maps"] = in_maps
    res = run_bass_kernel_spmd(nc, in_maps, list(range(N_CORES)))
    _CACHE["last_result"] = res
    out = np.concatenate([res.results[i]["out"] for i in range(N_CORES)], axis=0)
    return out.reshape(1, T, HID)


# revision 9
# speedup vs baseline: 1.1123x; 1.1123x over previous
"""GQA attention (llama3-style RoPE, causal) on 8 trn2 NeuronCores.

Sharding: tensor-parallel over KV-head groups. Core i owns kv-head i and its
4 query heads: wq[:, i*512:(i+1)*512], wk/wv[:, i*128:(i+1)*128], and the
matching row-slice wo[i*512:(i+1)*512, :]. After the partial o_proj, a
ReduceScatter(add) over the sequence dim leaves core i with output rows
[i*256, (i+1)*256); the host concatenates the shards.

On-device layout (per core): everything is computed transposed-by-design so
no PE transposes are needed in the hot path:
  qT/kT  [d=128, T]   = wq/wk-tile.T @ xT          (lhsT=weight, rhs=xT)
  vT     [d, T]       -> PE-transposed to v [T, d] (16 small transposes)
  sT     [tk, tq]     = k @ qT                     (lhsT=kT-tile, rhs=qT)
  pT     = exp(sT/sqrt(d)) * causal-mask
  l(row) [1, tq]      = ones.T @ pT                (M=1 matmul, PSUM-accum)
  oT     [d, tq]      = v.T @ pT                   (PSUM-accum over tk)
  o_norm = oT * (1/l) partition-broadcast
  partial[t, :]       = out_heads @ wo_i           (lhsT=oT-tile, rhs=wo)
All matmuls run as float32r (1 cycle/row vs 4 for fp32; ~1e-4 rel err).
"""

import numpy as np

H, KV, HD, HID = 32, 8, 128, 4096
T = 2048
N_CORES = 8
QH = H // KV            # 4 query heads per core
DQ = QH * HD            # 512
KT = HID // 128         # 32 contraction tiles for projections
TN = T // 128           # 16 sequence tiles
G = 4                   # tq groups of 512
GW = T // G             # 512
TS = T // N_CORES       # 256 output rows per core after ReduceScatter

THETA, FACTOR, HI_FF, LO_FF, ORIG_MAX = 500000.0, 8.0, 4.0, 1.0, 8192

_CACHE = {}


def _rope_tables():
    inv = 1.0 / (THETA ** (np.arange(0, HD, 2, dtype=np.float64) / HD))
    wavelen = 2.0 * np.pi / inv
    low_wl = ORIG_MAX / LO_FF
    high_wl = ORIG_MAX / HI_FF
    smooth = (ORIG_MAX / wavelen - LO_FF) / (HI_FF - LO_FF)
    scaled = np.where(wavelen > low_wl, inv / FACTOR, inv)
    mid = (wavelen <= low_wl) & (wavelen >= high_wl)
    scaled = np.where(mid, (1 - smooth) * inv / FACTOR + smooth * inv, scaled)
    inv32 = scaled.astype(np.float32)
    pos = np.arange(T, dtype=np.float32)
    freqs = pos[:, None] * inv32[None, :]          # [T, 64]
    emb = np.concatenate([freqs, freqs], axis=-1)  # [T, 128]
    cosT = np.ascontiguousarray(np.cos(emb).T)     # [128, T]
    sinT = np.ascontiguousarray(np.sin(emb).T)
    return cosT, sinT


def _causal_masks():
    # pT tile is [tk(part) 128, tq(free) 512]; within a tq-group the diagonal
    # tile sits at block v (=tk_tile - 4*g). keep where tq >= tk.
    tri = np.triu(np.ones((128, 128), dtype=np.float32))
    masks = np.zeros((4, 128, 512), dtype=np.float32)
    for v in range(4):
        for c in range(4):
            if c > v:
                masks[v, :, c * 128:(c + 1) * 128] = 1.0
            elif c == v:
                masks[v, :, c * 128:(c + 1) * 128] = tri
    return masks


def _build_program():
    import concourse.bacc as bacc
    import concourse.mybir as mybir
    from concourse.tile import TileContext

    f32 = mybir.dt.float32
    f32r = mybir.dt.float32r
    EXPF = mybir.ActivationFunctionType.Exp

    nc = bacc.Bacc("TRN2", target_bir_lowering=False, debug=False,
                   num_devices=N_CORES)

    xT = nc.dram_tensor("xT", [HID, T], f32, kind="ExternalInput")
    wqd = nc.dram_tensor("wq", [HID, DQ], f32, kind="ExternalInput")
    wkd = nc.dram_tensor("wk", [HID, HD], f32, kind="ExternalInput")
    wvd = nc.dram_tensor("wv", [HID, HD], f32, kind="ExternalInput")
    wod = nc.dram_tensor("wo", [DQ, HID], f32, kind="ExternalInput")
    cosd = nc.dram_tensor("cosT", [HD, T], f32, kind="ExternalInput")
    sind = nc.dram_tensor("sinT", [HD, T], f32, kind="ExternalInput")
    maskd = nc.dram_tensor("masks", [4, HD, GW], f32, kind="ExternalInput")
    identd = nc.dram_tensor("ident", [128, 128], f32, kind="ExternalInput")
    onesd = nc.dram_tensor("ones", [128, 1], f32, kind="ExternalInput")
    outd = nc.dram_tensor("out", [TS, HID], f32, kind="ExternalOutput")

    partial = nc.dram_tensor("partial", [T, HID], f32)
    rs_out = nc.dram_tensor("rs_out", [TS, HID], f32)

    def r(ap):
        return ap.bitcast(f32r)

    with TileContext(nc) as tc:
        with (
            tc.tile_pool(name="const", bufs=1) as cpool,
            tc.tile_pool(name="qkv", bufs=1) as qkv,
            tc.tile_pool(name="obuf", bufs=4) as obp,
        ):
            # ---- constants (DMAs deferred into g==0 to unblock first matmuls) ----
            cos = cpool.tile([HD, T], f32, tag="cos")
            sin = cpool.tile([HD, T], f32, tag="sin")
            ident = cpool.tile([128, 128], f32r, tag="ident")
            ones = cpool.tile([128, 1], f32r, tag="ones")
            # persistent activations
            qT = [qkv.tile([128, T], f32r, tag=f"qT{h}", name=f"qT{h}") for h in range(QH)]
            kTt = qkv.tile([128, T], f32r, tag="kT")
            vsb = qkv.tile([128, T], f32r, tag="vsb")  # v tiles side by side

            # ---- phase B: projections + RoPE (+ v transpose) ----
            with (
                tc.tile_pool(name="wq", bufs=1) as wqp,
                tc.tile_pool(name="stream", bufs=3) as stp,
                tc.tile_pool(name="tmp", bufs=2) as tmp,
                tc.tile_pool(name="ppsum", bufs=1, space="PSUM") as pps,
                tc.tile_pool(name="trpsum", bufs=2, space="PSUM") as trp,
            ):
                wq_t = [None] * KT

                for g in range(G):
                    gs = slice(g * GW, (g + 1) * GW)
                    qps = [pps.tile([128, GW], f32, tag=f"qp{h}", name=f"qp{h}_{g}") for h in range(QH)]
                    kps = pps.tile([128, GW], f32, tag="kp")
                    vps = pps.tile([128, GW], f32, tag="vp")
                    for k in range(KT):
                        xt = stp.tile([128, GW], f32r, tag="xt")
                        nc.sync.dma_start(xt[:], r(xT[k * 128:(k + 1) * 128, gs]))
                        if wq_t[k] is None:
                            wt = wqp.tile([128, DQ], f32r, tag=f"wq{k}", name=f"wq{k}")
                            nc.sync.dma_start(wt[:], r(wqd[k * 128:(k + 1) * 128, :]))
                            wq_t[k] = wt
                        wkt = stp.tile([128, HD], f32r, tag="wkt")
                        nc.sync.dma_start(wkt[:], r(wkd[k * 128:(k + 1) * 128, :]))
                        wvt = stp.tile([128, HD], f32r, tag="wvt")
                        nc.sync.dma_start(wvt[:], r(wvd[k * 128:(k + 1) * 128, :]))
                        st = (k == 0)
                        sp = (k == KT - 1)
                        for h in range(QH):
                            nc.tensor.matmul(qps[h][:], wq_t[k][:, h * 128:(h + 1) * 128],
                                             xt[:], start=st, stop=sp)
                        nc.tensor.matmul(kps[:], wkt[:], xt[:], start=st, stop=sp)
                        nc.tensor.matmul(vps[:], wvt[:], xt[:], start=st, stop=sp)

                    if g == 0:
                        nc.sync.dma_start(cos[:], cosd[:])
                        nc.sync.dma_start(sin[:], sind[:])
                        nc.sync.dma_start(ident[:], r(identd[:]))
                        nc.sync.dma_start(ones[:], r(onesd[:]))

                    # RoPE drain for q heads and k; v transpose
                    for h in range(QH + 1):
                        src = qps[h] if h < QH else kps
                        dst = qT[h] if h < QH else kTt
                        t1 = tmp.tile([128, GW], f32, tag="t1")
                        nc.vector.tensor_mul(t1[:], src[:], cos[:, gs])
                        rot = tmp.tile([128, GW], f32, tag="rot")
                        nc.scalar.mul(rot[0:64, :], src[64:128, :], -1.0)
                        nc.scalar.copy(rot[64:128, :], src[0:64, :])
                        rot2 = tmp.tile([128, GW], f32, tag="rot2")
                        nc.vector.tensor_mul(rot2[:], rot[:], sin[:, gs])
                        nc.vector.tensor_add(dst[:, gs].bitcast(f32r), t1[:], rot2[:])

                    vTt = tmp.tile([128, GW], f32r, tag="vT")
                    nc.vector.tensor_copy(vTt[:], vps[:])
                    for ts in range(4):
                        tp = trp.tile([128, 128], f32r, tag="trp")
                        nc.tensor.transpose(tp[:], vTt[:, ts * 128:(ts + 1) * 128], ident[:])
                        nc.vector.tensor_copy(
                            vsb[:, (4 * g + ts) * 128:(4 * g + ts + 1) * 128].bitcast(f32r),
                            tp[:])

            # ---- phases C+D+E interleaved: attention (g-outer) -> o_proj for
            # the finished tq block -> chunked ReduceScatter (overlaps compute) ----
            scale = float(1.0 / np.sqrt(HD))
            with (
                tc.tile_pool(name="mask", bufs=1) as mpool,
                tc.tile_pool(name="oT", bufs=1) as otp,
                tc.tile_pool(name="wor", bufs=1) as worp,
                tc.tile_pool(name="wos", bufs=2) as wop,
                tc.tile_pool(name="pt", bufs=4) as ptp,
                tc.tile_pool(name="norm", bufs=2) as nrm,
                tc.tile_pool(name="apsum", bufs=2, space="PSUM") as aps,
                tc.tile_pool(name="apsum1", bufs=1, space="PSUM") as aps1,
                tc.tile_pool(name="opsum", bufs=4, space="PSUM") as opsum,
            ):
                mtiles = []
                for v in range(4):
                    mt = mpool.tile([HD, GW], f32r, tag=f"mask{v}", name=f"mask{v}")
                    nc.sync.dma_start(mt[:], r(maskd[v]))
                    mtiles.append(mt)
                oT = [otp.tile([128, T], f32r, tag=f"oT{h}", name=f"oT{h}") for h in range(QH)]
                # wo: n-chunks 0..3 resident, 4..7 streamed per g
                wo_res = []
                for f in range(QH):
                    wc = worp.tile([128, 2048], f32r, tag=f"wor{f}", name=f"wor{f}")
                    nc.sync.dma_start(wc[:], r(wod[f * 128:(f + 1) * 128, 0:2048]))
                    wo_res.append(wc)

                for g in range(G):
                    gs = slice(g * GW, (g + 1) * GW)
                    nj = 4 * g + 4
                    for h in range(QH):
                        ops_ = aps1.tile([128, GW], f32, tag="op")
                        lps = aps1.tile([1, GW], f32, tag="lp")
                        for j in range(nj):
                            sps = aps.tile([128, GW], f32, tag="sp")
                            nc.tensor.matmul(sps[:], kTt[:, j * 128:(j + 1) * 128],
                                             qT[h][:, gs], start=True, stop=True)
                            pt = ptp.tile([128, GW], f32r, tag="pt")
                            nc.scalar.activation(pt[:], sps[:], EXPF, scale=scale)
                            if j >= 4 * g:
                                nc.vector.tensor_mul(pt[:], pt[:], mtiles[j - 4 * g][:])
                            nc.tensor.matmul(lps[:], ones[:], pt[:],
                                             start=(j == 0), stop=(j == nj - 1))
                            nc.tensor.matmul(ops_[:], vsb[:, j * 128:(j + 1) * 128],
                                             pt[:], start=(j == 0), stop=(j == nj - 1))
                        ls = nrm.tile([1, GW], f32, tag="ls")
                        nc.vector.reciprocal(ls[:], lps[:])
                        lb = nrm.tile([128, GW], f32, tag="lb")
                        nc.gpsimd.partition_broadcast(lb[:], ls[:])
                        nc.vector.tensor_mul(oT[h][:, gs].bitcast(f32r), ops_[:], lb[:])

                    # o_proj for this tq block (t tiles 4g..4g+3)
                    for n in range(HID // 512):
                        if n < 4:
                            wo_c = [wo_res[f][:, n * 512:(n + 1) * 512] for f in range(QH)]
                        else:
                            wo_c = []
                            for f in range(QH):
                                wc = wop.tile([128, 512], f32r, tag=f"woc{f}",
                                              name=f"woc{f}_{g}_{n}")
                                nc.sync.dma_start(
                                    wc[:],
                                    r(wod[f * 128:(f + 1) * 128, n * 512:(n + 1) * 512]))
                                wo_c.append(wc[:])
                        for t in range(4 * g, 4 * g + 4):
                            ops_ = opsum.tile([128, 512], f32, tag="oproj")
                            for f in range(QH):
                                nc.tensor.matmul(ops_[:], oT[f][:, t * 128:(t + 1) * 128],
                                                 wo_c[f],
                                                 start=(f == 0), stop=(f == QH - 1))
                            ob = obp.tile([128, 512], f32, tag="ob")
                            nc.vector.tensor_copy(ob[:], ops_[:])
                            nc.sync.dma_start(
                                partial[t * 128:(t + 1) * 128, n * 512:(n + 1) * 512],
                                ob[:])

                    # chunked ReduceScatter for rows [g*512, (g+1)*512):
                    # core i receives rows g*512 + i*64 .. +64 -> outd[g*64:(g+1)*64]
                    nc.gpsimd.collective_compute(
                        "ReduceScatter", mybir.AluOpType.add,
                        replica_groups=[list(range(N_CORES))],
                        ins=[partial[g * GW:(g + 1) * GW, :]],
                        outs=[rs_out[g * (GW // N_CORES):(g + 1) * (GW // N_CORES), :]],
                    )
                    nc.sync.dma_start(
                        outd[g * (GW // N_CORES):(g + 1) * (GW // N_CORES), :],
                        rs_out[g * (GW // N_CORES):(g + 1) * (GW // N_CORES), :])

    nc.compile()
    return nc


def _get_program():
    if "nc" not in _CACHE:
        _CACHE["nc"] = _build_program()
    return _CACHE["nc"]


def kernel(x, wq, wk, wv, wo):
    from concourse.bass_utils import run_bass_kernel_spmd

    nc = _get_program()

    x2 = np.asarray(x, dtype=np.float32).reshape(T, HID)
    xT = np.ascontiguousarray(x2.T)
    cosT, sinT = _rope_tables()
    masks = _causal_masks()
    ident = np.eye(128, dtype=np.float32)
    ones = np.ones((128, 1), dtype=np.float32)

    wq = np.asarray(wq, dtype=np.float32)
    wk = np.asarray(wk, dtype=np.float32)
    wv = np.asarray(wv, dtype=np.float32)
    wo = np.asarray(wo, dtype=np.float32)

    in_maps = []
    for i in range(N_CORES):
        in_maps.append({
            "xT": xT,
            "wq": np.ascontiguousarray(wq[:, i * DQ:(i + 1) * DQ]),
            "wk": np.ascontiguousarray(wk[:, i * HD:(i + 1) * HD]),
            "wv": np.ascontiguousarray(wv[:, i * HD:(i + 1) * HD]),
            "wo": np.ascontiguousarray(wo[i * DQ:(i + 1) * DQ, :]),
            "cosT": cosT,
            "sinT": sinT,
            "masks": masks,
            "ident": ident,
            "ones": ones,
        })

    _CACHE["last_in_maps"] = in_maps
    res = run_bass_kernel_spmd(nc, in_maps, list(range(N_CORES)))
    _CACHE["last_result"] = res
    # chunked RS layout: core i's rows [g*64:(g+1)*64] are global rows
    # g*512 + i*64 .. g*512 + (i+1)*64
    W8 = GW // N_CORES
    out = np.empty((T, HID), dtype=np.float32)
    for i in range(N_CORES):
        oi = res.results[i]["out"]
        for g in range(G):
            out[g * GW + i * W8:g * GW + (i + 1) * W8] = oi[g * W8:(g + 1) * W8]
    return out.reshape(1, T, HID)
